# revision 29
# baseline (speedup 1.0000x reference)
"""
Binary Conv2d (BBCU-style) block on 8 Trainium2 NeuronCores — v3.

Computation (per reference):
    z  = sign(x + move0_bias)                    # binarized activations in {-1,1}
    bw = scale[o] * sign(W)                      # binarized weights
    y  = conv3x3(z, bw, pad=1)
    y  = prelu(y + pr_bias0, a) + pr_bias1 + x   # RPReLU + identity

Design:
  * bf16 I/O. Host sends x_dev = bf16(x + move0_bias) pre-packed in the
    parity layout (64ch x row-parity partitions); every HBM DMA is a
    contiguous 1MB transfer. bf16 rounding cannot flip sign(x+b0) (the bias
    is added before rounding), so the binarization stays exact; only the
    identity add and the bf16 store round (~2^-8 rel, tolerance is 2e-2).
  * Binarization on DVE: z' = is_ge(x_dev, 0) in {0,1} fp8. The conv then
    computes S' = sum(w * z'); since z = 2z'-1, the epilogue folds it back
    via scale_eff = 2*scale and bias_eff = pb0 - scale*rowsum(sign(w)).
    Zero-padding taps are stored as 0.5 so they contribute 2*0.5-1 = 0.
  * Conv = 3 fp8 DoubleRow matmuls (K=256) per output row-pair: k-tile 0 is
    the "type-1" taps (within-pair rows), k-tile 1 the "type-2" taps (the
    cross-pair row, supplied as a shifted B plane). The zs layout is
    chunk-local [A block 4352B | B block 4352B] so the k-tile AP step is
    4352 bytes (16B-aligned, < 2^15). The B plane comes packed from the
    host (its content is the sign plane shifted +/-1 row-pair by parity),
    so no on-device shuffle is needed and chunks are fully independent.
  * Epilogue: ACT Prelu over 4-bank [128,2048] PSUM tiles -> bf16 gt, then
    DVE: gt2 = gt + (pr_bias1 - move0_bias); out = gt2 + x_dev; DMA out.

Sharding: data-parallel over batch, 2 images per core.
"""

import os
from contextlib import ExitStack

import numpy as np

import ml_dtypes

import concourse.bass as bass
import concourse.mybir as mybir
import concourse.tile as tile
from concourse.bass_utils import run_bass_kernel_spmd

# ---------------------------------------------------------------------------
# Workaround: the in-container walrus rejects instructions carrying more than
# 1 semaphore wait; move excess waits onto NoOp instructions inserted just
# before the carrier (same engine => program order preserves happens-before).
# ---------------------------------------------------------------------------
_MAX_WAITS = 1


def _split_sync_waits(mod: dict, max_waits: int = _MAX_WAITS) -> dict:
    for fn in mod.get("functions", []):
        for bb in fn.get("blocks", []):
            out = []
            for ins in bb.get("instructions", []):
                si = ins.get("sync_info")
                waits = (si or {}).get("on_wait") or []
                if len(waits) > max_waits:
                    extra, keep = waits[:-max_waits], waits[-max_waits:]
                    for i in range(0, len(extra), max_waits):
                        out.append({
                            "debug": ins.get("debug", 0),
                            "engine": ins["engine"],
                            "ins": [],
                            "name": f"{ins['name']}_ws{i}",
                            "opcode": "NoOp",
                            "outs": [],
                            "sync_info": {
                                "on_update": [],
                                "on_wait": extra[i:i + max_waits],
                            },
                        })
                    si["on_wait"] = keep
                out.append(ins)
            bb["instructions"] = out
    return mod


def _dedup_ldweights(mod: dict) -> dict:
    """Drop PE Ldweights whose weights operand matches the previous one.

    Bass splits every self-loading matmul into an explicit Ldweights +
    bare Matmult pair (the matmul's waits ride on the Ldweights).
    Consecutive matmuls that share a stationary only need the first load;
    dropped loads hand their waits to the following instruction.
    """
    import orjson

    for fn in mod.get("functions", []):
        for bb in fn.get("blocks", []):
            out = []
            last_key = None
            pend_waits = []
            for ins in bb.get("instructions", []):
                if ins.get("engine") != "PE":
                    out.append(ins)
                    continue
                op = ins.get("opcode")
                if op == "Ldweights":
                    si = ins.get("sync_info") or {}
                    if (si.get("on_update") or []) == []:
                        key = orjson.dumps(
                            [ins.get("ins"), ins.get("perf_mode"),
                             ins.get("tile_position"), ins.get("tile_size")])
                        if key == last_key:
                            pend_waits.extend(si.get("on_wait") or [])
                            continue
                        last_key = key
                elif op != "Matmult":
                    last_key = None
                if pend_waits:
                    si = ins.get("sync_info")
                    if si is None:
                        si = {"on_update": [], "on_wait": []}
                        ins["sync_info"] = si
                    si["on_wait"] = list(si.get("on_wait") or []) + pend_waits
                    pend_waits = []
                out.append(ins)
            assert not pend_waits
            bb["instructions"] = out
    return mod


_orig_to_json_bytes = bass.Bass.to_json_bytes


def _to_json_bytes_split(self):
    import orjson

    mod = orjson.loads(_orig_to_json_bytes(self))
    if os.environ.get("BBCU_DEDUP", "1") != "0":
        mod = _dedup_ldweights(mod)
    return orjson.dumps(_split_sync_waits(mod))


bass.Bass.to_json_bytes = _to_json_bytes_split

# Optionally let walrus dedup per-matmul LDWEIGHTS (its ldw-opt pass is
# hardcoded off in bass_utils).
if os.environ.get("BBCU_LDWOPT", "0") != "0":
    import concourse.bass_utils as _bu

    _orig_run_command = _bu.run_command

    def _run_command_ldwopt(argv, **kw):
        if isinstance(argv, list):
            argv = ["--enable-ldw-opt=true" if a == "--enable-ldw-opt=false"
                    else a for a in argv]
        return _orig_run_command(argv, **kw)

    _bu.run_command = _run_command_ldwopt

F32 = mybir.dt.float32
BF16 = mybir.dt.bfloat16
FP8 = mybir.dt.float8e4
NP_FP8 = ml_dtypes.float8_e4m3
NP_BF16 = ml_dtypes.bfloat16
AL = mybir.AluOpType

# consts column indices (S' = conv of 0/1 plane; scale_eff = 2*scale,
# pb0_eff = pb0 - scale*rowsum)
C_SC = 0      # scale_eff (prelu activation scale)
C_PB0 = 1     # pb0_eff   (prelu activation bias)
C_AL = 2      # prelu alpha
C_FIN = 3     # pr_bias1 - move0_bias (final add)
C_RS = 4      # (1-a)*scale_eff        (relu path: ACT scale)
C_RB = 5      # (1-a)*pb0_eff          (relu path: ACT bias)
C_VS = 6      # a*scale_eff            (relu path: stt scalar)
C_RF = 7      # a*pb0_eff + pr_bias1 - move0_bias (relu path: final add)
# +-1 encoding variants (for chunks whose sign runs on ACT as Sign)
C2_SC = 8     # scale
C2_PB0 = 9    # pb0
C2_RS = 10
C2_RB = 11
C2_VS = 12
C2_RF = 13
NCOL = 14

SLOT = 272      # bytes per row-pair slot (16-aligned, >= 258)
PAD = np.float32(0.5)  # pad value: contributes 2*0.5-1 = 0 after the fold


_PM_SET = frozenset(
    int(t) for t in os.environ.get("BBCU_PM", "").split(",") if t != "")


def _is_pm(k: int) -> bool:
    # blocks whose sign runs on ACT with +-1 encoding (offloads the DVE)
    return k in _PM_SET


def _build3(Bc: int, H: int, W: int, C: int, G: int, look: int,
            use_prelu: bool = True):
    """Per-core Bass module. Chunk = G rows; parity layout 64ch x 2 parities."""
    assert C == 64 and W == 256
    assert H % G == 0 and G % 4 == 0
    P = G // 2             # row-pairs per chunk
    NCH = H // G           # chunks per image
    ABLK = P * SLOT        # 4352
    NCHT = Bc * NCH        # total chunks per core

    nc = bass.Bass()
    xd = nc.declare_dram_parameter("x", [NCHT, 128, P * W], BF16, isOutput=False)
    bd = nc.declare_dram_parameter("zb", [NCHT, 128, ABLK], FP8, isOutput=False)
    wd = nc.declare_dram_parameter("wp", [128, 768], FP8, isOutput=False)
    cd = nc.declare_dram_parameter("cv", [128, NCOL], F32, isOutput=False)
    cbd = nc.declare_dram_parameter("cvb", [128, NCOL], BF16, isOutput=False)
    yd = nc.declare_dram_parameter("y", [NCHT, 128, P * W], BF16, isOutput=True)

    with ExitStack() as ctx:
        tc = ctx.enter_context(tile.TileContext(nc))
        cpool = ctx.enter_context(tc.tile_pool(name="const", bufs=1))
        zpool = ctx.enter_context(tc.tile_pool(name="zs", bufs=1))
        XB = int(os.environ.get("BBCU_XB", "4"))
        xpool = ctx.enter_context(tc.tile_pool(name="xt", bufs=look + XB))
        gpool = ctx.enter_context(tc.tile_pool(name="gt", bufs=4 if G <= 32 else 2))
        g2pool = ctx.enter_context(tc.tile_pool(name="gt2", bufs=3))
        pspool = ctx.enter_context(tc.tile_pool(name="ps", bufs=2, space="PSUM"))

        # --- resident constants ---
        wsb = cpool.tile([128, 768], FP8)
        nc.sync.dma_start(wsb[:], wd[:])
        cvs = cpool.tile([128, NCOL], F32)
        nc.sync.dma_start(cvs[:], cd[:])
        cvb = cpool.tile([128, NCOL], BF16)
        nc.sync.dma_start(cvb[:], cbd[:])
        wv = wsb[:].rearrange("k (d z m) -> k d z m", d=3, m=128)

        # zs: per image-chunk k, [A block | B block]; A slot j = 0/1 plane of
        # row-pair kP+j (even rows parts 0:64, odd 64:128), B slot j = the
        # type-2 cross-pair rows (host-packed, halos and pads included).
        zsall = zpool.tile([128, NCH * 2 * ABLK], FP8)
        zv = zsall[:].rearrange("p (k ab s c) -> p k ab s c", ab=2, s=P, c=SLOT)

        # one-time A column pads (B comes fully padded from the host):
        # 0.5 for 0/1-encoded blocks, 0.0 for +-1-encoded blocks
        for k in range(NCH):
            pv = 0.0 if _is_pm(k) else float(PAD)
            nc.gpsimd.memset(zv[:, k, 0, :, 0:1], pv)
            nc.gpsimd.memset(zv[:, k, 0, :, 257:272], pv)

        # PE pstate warmup: dummy matmuls on a scratch tile while the first
        # chunk's loads are in flight (results discarded; first real matmul
        # clears its bank with start=True).
        WARM = int(os.environ.get("BBCU_WARM", "6"))

        def load(cc):
            xt = xpool.tile([128, P * W], BF16, name=f"xt_{cc}", tag="xt")
            nc.sync.dma_start(xt[:], xd[cc])
            k = cc % NCH
            nc.sync.dma_start(
                zsall[:, (2 * k + 1) * ABLK:(2 * k + 2) * ABLK], bd[cc])
            if cc == 0 and WARM:
                # PE pstate warmup just-in-time: reads chunk 0's B block, so
                # it starts only once that load lands (right before the first
                # real matmuls). Results are discarded (start=True clears).
                wps = pspool.tile([128, 2048], F32, name="ps")
                for wi in range(WARM):
                    nc.tensor.matmul(
                        wps[:, (wi % 8) * 256:(wi % 8 + 1) * 256],
                        wv[:, 0],
                        zv[:, 0, 1, 0:2, 0:256],
                        start=True, stop=True,
                        perf_mode=mybir.MatmulPerfMode.DoubleRow,
                    )
            return xt

        def sign(cc, xt):
            k = cc % NCH
            if _is_pm(k):
                nc.scalar.activation(
                    zv[:, k, 0, :, 1:257],
                    xt[:].rearrange("p (s c) -> p s c", c=W),
                    mybir.ActivationFunctionType.Sign)
            else:
                nc.vector.tensor_scalar(
                    zv[:, k, 0, :, 1:257],
                    xt[:].rearrange("p (s c) -> p s c", c=W),
                    0.0, None, op0=AL.is_ge)

        MW = int(os.environ.get("BBCU_MW", "2"))  # output slots per matmul
        LDW = os.environ.get("BBCU_LDW", "0") != "0"  # dedup ldweights

        def conv(cc, xt):
            k = cc % NCH
            gt = gpool.tile([128, P * W], BF16, name=f"gt_{cc}", tag="gt")
            NT = P // 8
            pss = [pspool.tile([128, 2048], F32, name="ps")
                   for t in range(NT)]
            if LDW:
                # dwi-major over the whole chunk: the first matmul of each
                # dwi run self-loads the stationary; the rest reuse it
                # (ldweights=False suppresses the per-matmul LDWEIGHTS).
                for dwi in range(3):
                    first = True
                    for t in range(NT):
                        for q in range(8 // MW):
                            j = 8 * t + MW * q
                            mm = nc.tensor.matmul(
                                pss[t][:, q * MW * 256:(q + 1) * MW * 256],
                                wv[:, dwi],
                                zv[:, k, :, j:j + MW, dwi:dwi + 256],
                                start=(dwi == 0),
                                stop=(dwi == 2),
                                perf_mode=mybir.MatmulPerfMode.DoubleRow,
                            )
                            if not first:
                                mm.ins.ldweights = False
                            first = False
            else:
                for t in range(NT):
                    ps = pss[t]
                    # dwi-outer so consecutive matmuls share weights;
                    # accumulation groups are per bank.
                    for dwi in range(3):
                        for q in range(8 // MW):
                            j = 8 * t + MW * q
                            nc.tensor.matmul(
                                ps[:, q * MW * 256:(q + 1) * MW * 256],
                                wv[:, dwi],
                                zv[:, k, :, j:j + MW, dwi:dwi + 256],
                                start=(dwi == 0),
                                stop=(dwi == 2),
                                perf_mode=mybir.MatmulPerfMode.DoubleRow,
                            )
            pm = _is_pm(k)
            for t in range(NT):
                ps = pss[t]
                gslice = gt[:, t * 2048:(t + 1) * 2048]
                csc = C2_SC if pm else C_SC
                cpb = C2_PB0 if pm else C_PB0
                crs = C2_RS if pm else C_RS
                crb = C2_RB if pm else C_RB
                cvsq = C2_VS if pm else C_VS
                if use_prelu:
                    nc.scalar.activation(
                        gslice,
                        ps[:],
                        mybir.ActivationFunctionType.Prelu,
                        bias=cvs[:, cpb:cpb + 1],
                        scale=cvs[:, csc:csc + 1],
                        alpha=cvs[:, C_AL:C_AL + 1],
                    )
                else:
                    rt = g2pool.tile([128, 2048], F32, name="rt", tag="rt")
                    nc.scalar.activation(
                        rt[:],
                        ps[:],
                        mybir.ActivationFunctionType.Relu,
                        bias=cvs[:, crb:crb + 1],
                        scale=cvs[:, crs:crs + 1],
                    )
                    nc.vector.scalar_tensor_tensor(
                        gslice, ps[:], cvs[:, cvsq:cvsq + 1], rt[:],
                        op0=AL.mult, op1=AL.add)
            # y = (g + cF) + x_dev  (one non-aliased DVE stt, one 1MB store)
            if use_prelu:
                ccol = C_FIN
            else:
                ccol = C2_RF if pm else C_RF
            gt2 = g2pool.tile([128, P * W], BF16, name=f"g2_{cc}", tag="g2")
            if os.environ.get("BBCU_STTH", "0") != "0" and NT == 2:
                # halves: the first can run while ACT handles tile 1
                for t in range(NT):
                    lo, hi = t * 2048, (t + 1) * 2048
                    nc.vector.scalar_tensor_tensor(
                        gt2[:, lo:hi], gt[:, lo:hi], cvb[:, ccol:ccol + 1],
                        xt[:, lo:hi], op0=AL.add, op1=AL.add)
            else:
                nc.vector.scalar_tensor_tensor(
                    gt2[:], gt[:], cvb[:, ccol:ccol + 1], xt[:],
                    op0=AL.add, op1=AL.add)
            nc.sync.dma_start(yd[cc], gt2[:])

        CONV_FIRST = os.environ.get("BBCU_CF", "0") != "0"
        xts = {}
        for idx in range(NCHT):
            xts[idx] = load(idx)
            if CONV_FIRST:
                if idx >= look:
                    conv(idx - look, xts.pop(idx - look))
                sign(idx, xts[idx])
            else:
                sign(idx, xts[idx])
                if idx >= look:
                    conv(idx - look, xts.pop(idx - look))
        for idx in sorted(xts):
            conv(idx, xts.pop(idx))

    return nc


def _host_prep3(move0_bias, conv_weight, prelu_weight, pr_bias0, pr_bias1):
    """Pack weights into [128, 3*2*128] fp8 lhsT + constant vectors."""
    w = np.asarray(conv_weight, dtype=np.float32)          # [O, I, 3, 3]
    sw = np.sign(w).astype(np.float32)
    scale = np.mean(np.abs(w), axis=(1, 2, 3)).astype(np.float32)  # [O]
    a = np.asarray(prelu_weight, dtype=np.float32).reshape(64)
    pb0 = np.asarray(pr_bias0, dtype=np.float32).reshape(64)
    pb1 = np.asarray(pr_bias1, dtype=np.float32).reshape(64)
    b0 = np.asarray(move0_bias, dtype=np.float32).reshape(64)

    # lhsT[k, m]: k = pi*64 + ci, m = po*64 + co -> sw[co, ci, kh, kw]
    # ktile 0 (type-1): dh = [[0, -1], [1, 0]][pi][po]
    # ktile 1 (type-2): (pi0,po1) = +1, (pi1,po0) = -1
    swT = np.transpose(sw, (1, 0, 2, 3))  # [ci, co, kh, kw]
    wdr = np.zeros((128, 3, 2, 128), dtype=np.float32)
    for idw in range(3):
        kw = idw
        wdr[0:64, idw, 0, 0:64] = swT[:, :, 1, kw]      # even->even  dh=0
        wdr[0:64, idw, 0, 64:128] = swT[:, :, 0, kw]    # even->odd   dh=-1
        wdr[64:128, idw, 0, 0:64] = swT[:, :, 2, kw]    # odd->even   dh=+1
        wdr[64:128, idw, 0, 64:128] = swT[:, :, 1, kw]  # odd->odd    dh=0
        wdr[0:64, idw, 1, 64:128] = swT[:, :, 2, kw]    # row 2i+2 -> out 2i+1
        wdr[64:128, idw, 1, 0:64] = swT[:, :, 0, kw]    # row 2i-1 -> out 2i
    wp8 = wdr.reshape(128, 768).astype(NP_FP8)

    # rowsum over the full K=256 contraction and all 3 dw taps, per out col m
    rowsum = wdr.sum(axis=(0, 1, 2))                    # [128]
    sc2 = np.concatenate([scale, scale])                # [128]
    sc_eff = 2.0 * sc2
    pb0_eff = np.concatenate([pb0, pb0]) - sc2 * rowsum

    cv = np.zeros((128, NCOL), dtype=np.float32)
    aa = np.concatenate([a, a])
    fin = np.concatenate([pb1 - b0, pb1 - b0])
    cv[:, C_SC] = sc_eff
    cv[:, C_PB0] = pb0_eff
    cv[:, C_AL] = aa
    cv[:, C_FIN] = fin
    cv[:, C_RS] = (1.0 - aa) * sc_eff
    cv[:, C_RB] = (1.0 - aa) * pb0_eff
    cv[:, C_VS] = aa * sc_eff
    cv[:, C_RF] = aa * pb0_eff + fin
    pb0_pm = np.concatenate([pb0, pb0])
    cv[:, C2_SC] = sc2
    cv[:, C2_PB0] = pb0_pm
    cv[:, C2_RS] = (1.0 - aa) * sc2
    cv[:, C2_RB] = (1.0 - aa) * pb0_pm
    cv[:, C2_VS] = aa * sc2
    cv[:, C2_RF] = aa * pb0_pm + fin
    return wp8, cv


def _pack_x(x, b0, G=32):
    """x [B,C,H,W] f32 -> [B, NCH, 128, (G/2)*W] bf16 of bf16(x + b0)."""
    B, C, H, W = x.shape
    P = G // 2
    NCH = H // G
    t = (x + b0.reshape(1, C, 1, 1)).astype(NP_BF16)
    v = t.reshape(B, C, NCH, P, 2, W).transpose(0, 2, 4, 1, 3, 5)
    return np.ascontiguousarray(v.reshape(B, NCH, 2 * C, P * W))


def _pack_zb(x, b0, G=32):
    """Host B plane: [B, NCH, 128, P*SLOT] fp8 of the shifted 0/1 plane.

    B slot j: even parts = 0/1 of even row of pair kP+j+1;
              odd parts  = 0/1 of odd row  of pair kP+j-1; halos/pads 0.5.
    """
    B, C, H, W = x.shape
    P = G // 2
    NCH = H // G
    NPAIR = H // 2
    z = ((x + b0.reshape(1, C, 1, 1)) >= 0).astype(np.float32)  # {0,1}
    zp = z.reshape(B, C, NPAIR, 2, W)  # [B, C, pair, parity, W]
    halo = np.full((B, C, 1, W), PAD, dtype=np.float32)
    even_sh = np.concatenate([zp[:, :, 1:, 0, :], halo], axis=2)   # pair+1 even
    odd_sh = np.concatenate([halo, zp[:, :, :-1, 1, :]], axis=2)   # pair-1 odd
    out = np.full((B, NCH, 2, C, P, SLOT), PAD, dtype=np.float32)
    ev = even_sh.reshape(B, C, NCH, P, W).transpose(0, 2, 1, 3, 4)
    od = odd_sh.reshape(B, C, NCH, P, W).transpose(0, 2, 1, 3, 4)
    out[:, :, 0, :, :, 1:257] = ev
    out[:, :, 1, :, :, 1:257] = od
    # +-1 encoded blocks: map {0,1,0.5(pad)} -> {-1,1,0}
    for k in range(NCH):
        if _is_pm(k):
            out[:, k] = 2.0 * out[:, k] - 1.0
    return np.ascontiguousarray(
        out.reshape(B, NCH, 128, P * SLOT).astype(NP_FP8))


def _unpack_y(yp, B, C, H, W, G=32):
    P = G // 2
    NCH = H // G
    v = yp.reshape(B, NCH, 2, C, P, W).transpose(0, 3, 1, 4, 2, 5)
    return v.reshape(B, C, H, W).astype(np.float32)


_NC_CACHE: dict = {}


def _get_nc3(key, *args):
    if key not in _NC_CACHE:
        _NC_CACHE[key] = _build3(*args)
    return _NC_CACHE[key]


def _make_in_maps(inputs, NCORES=8, G=32):
    x = np.asarray(inputs["x"], dtype=np.float32)
    B, C, H, W = x.shape
    Bc = B // NCORES
    NCH = H // G
    P = G // 2
    b0 = np.asarray(inputs["move0_bias"], dtype=np.float32)
    wp8, cv = _host_prep3(
        inputs["move0_bias"], inputs["conv_weight"], inputs["prelu_weight"],
        inputs["pr_bias0"], inputs["pr_bias1"])
    xp = _pack_x(x, b0, G)
    zb = _pack_zb(x, b0, G)
    in_maps = [
        {
            "x": np.ascontiguousarray(
                xp[i * Bc:(i + 1) * Bc].reshape(Bc * NCH, 128, P * W)),
            "zb": np.ascontiguousarray(
                zb[i * Bc:(i + 1) * Bc].reshape(Bc * NCH, 128, P * SLOT)),
            "wp": wp8,
            "cv": cv,
            "cvb": cv.astype(NP_BF16),
        }
        for i in range(NCORES)
    ]
    return in_maps, (B, C, H, W, Bc)


def kernel(x, move0_bias, conv_weight, prelu_weight, pr_bias0, pr_bias1):
    inputs = dict(x=x, move0_bias=move0_bias, conv_weight=conv_weight,
                  prelu_weight=prelu_weight, pr_bias0=pr_bias0,
                  pr_bias1=pr_bias1)
    NCORES = 8
    G = int(os.environ.get("BBCU_G", "32"))
    look = int(os.environ.get("BBCU_LOOK", "2" if G <= 32 else "1"))
    use_prelu = os.environ.get("BBCU_PRELU", "1") != "0"

    in_maps, (B, C, H, W, Bc) = _make_in_maps(inputs, NCORES, G)
    key = (Bc, H, W, C, G, look, use_prelu)
    nc = _get_nc3(key, Bc, H, W, C, G, look, use_prelu)

    res = run_bass_kernel_spmd(nc, in_maps, core_ids=list(range(NCORES)))
    yps = np.stack([res.results[i]["y"] for i in range(NCORES)], axis=0)
    return _unpack_y(yps, B, C, H, W, G)



# revision 30
# speedup vs baseline: 1.1578x; 1.1578x over previous
"""
Binary Conv2d (BBCU-style) block on 8 Trainium2 NeuronCores — v3.

Computation (per reference):
    z  = sign(x + move0_bias)                    # binarized activations in {-1,1}
    bw = scale[o] * sign(W)                      # binarized weights
    y  = conv3x3(z, bw, pad=1)
    y  = prelu(y + pr_bias0, a) + pr_bias1 + x   # RPReLU + identity

Design:
  * bf16 I/O. Host sends x_dev = bf16(x + move0_bias) pre-packed in the
    parity layout (64ch x row-parity partitions); every HBM DMA is a
    contiguous 1MB transfer. bf16 rounding cannot flip sign(x+b0) (the bias
    is added before rounding), so the binarization stays exact; only the
    identity add and the bf16 store round (~2^-8 rel, tolerance is 2e-2).
  * Binarization on DVE: z' = is_ge(x_dev, 0) in {0,1} fp8. The conv then
    computes S' = sum(w * z'); since z = 2z'-1, the epilogue folds it back
    via scale_eff = 2*scale and bias_eff = pb0 - scale*rowsum(sign(w)).
    Zero-padding taps are stored as 0.5 so they contribute 2*0.5-1 = 0.
  * Conv = 3 fp8 DoubleRow matmuls (K=256) per output row-pair: k-tile 0 is
    the "type-1" taps (within-pair rows), k-tile 1 the "type-2" taps (the
    cross-pair row, supplied as a shifted B plane). The zs layout is
    chunk-local [A block 4352B | B block 4352B] so the k-tile AP step is
    4352 bytes (16B-aligned, < 2^15). The B plane comes packed from the
    host (its content is the sign plane shifted +/-1 row-pair by parity),
    so no on-device shuffle is needed and chunks are fully independent.
  * Epilogue: ACT Prelu over 4-bank [128,2048] PSUM tiles -> bf16 gt, then
    DVE: gt2 = gt + (pr_bias1 - move0_bias); out = gt2 + x_dev; DMA out.

Sharding: data-parallel over batch, 2 images per core.
"""

import os
from contextlib import ExitStack

import numpy as np

import ml_dtypes

import concourse.bass as bass
import concourse.mybir as mybir
import concourse.tile as tile
from concourse.bass_utils import run_bass_kernel_spmd

# ---------------------------------------------------------------------------
# Workaround: the in-container walrus rejects instructions carrying more than
# 1 semaphore wait; move excess waits onto NoOp instructions inserted just
# before the carrier (same engine => program order preserves happens-before).
# ---------------------------------------------------------------------------
_MAX_WAITS = 1


def _split_sync_waits(mod: dict, max_waits: int = _MAX_WAITS) -> dict:
    for fn in mod.get("functions", []):
        for bb in fn.get("blocks", []):
            out = []
            for ins in bb.get("instructions", []):
                si = ins.get("sync_info")
                waits = (si or {}).get("on_wait") or []
                if len(waits) > max_waits:
                    extra, keep = waits[:-max_waits], waits[-max_waits:]
                    for i in range(0, len(extra), max_waits):
                        out.append({
                            "debug": ins.get("debug", 0),
                            "engine": ins["engine"],
                            "ins": [],
                            "name": f"{ins['name']}_ws{i}",
                            "opcode": "NoOp",
                            "outs": [],
                            "sync_info": {
                                "on_update": [],
                                "on_wait": extra[i:i + max_waits],
                            },
                        })
                    si["on_wait"] = keep
                out.append(ins)
            bb["instructions"] = out
    return mod


def _dedup_ldweights(mod: dict) -> dict:
    """Drop PE Ldweights whose weights operand matches the previous one.

    Bass splits every self-loading matmul into an explicit Ldweights +
    bare Matmult pair (the matmul's waits ride on the Ldweights).
    Consecutive matmuls that share a stationary only need the first load;
    dropped loads hand their waits to the following instruction.
    """
    import orjson

    for fn in mod.get("functions", []):
        for bb in fn.get("blocks", []):
            out = []
            last_key = None
            pend_waits = []
            for ins in bb.get("instructions", []):
                if ins.get("engine") != "PE":
                    out.append(ins)
                    continue
                op = ins.get("opcode")
                if op == "Ldweights":
                    si = ins.get("sync_info") or {}
                    if (si.get("on_update") or []) == []:
                        key = orjson.dumps(
                            [ins.get("ins"), ins.get("perf_mode"),
                             ins.get("tile_position"), ins.get("tile_size")])
                        if key == last_key:
                            pend_waits.extend(si.get("on_wait") or [])
                            continue
                        last_key = key
                elif op != "Matmult":
                    last_key = None
                if pend_waits:
                    si = ins.get("sync_info")
                    if si is None:
                        si = {"on_update": [], "on_wait": []}
                        ins["sync_info"] = si
                    si["on_wait"] = list(si.get("on_wait") or []) + pend_waits
                    pend_waits = []
                out.append(ins)
            assert not pend_waits
            bb["instructions"] = out
    return mod


_orig_to_json_bytes = bass.Bass.to_json_bytes


def _to_json_bytes_split(self):
    import orjson

    mod = orjson.loads(_orig_to_json_bytes(self))
    if os.environ.get("BBCU_DEDUP", "1") != "0":
        mod = _dedup_ldweights(mod)
    return orjson.dumps(_split_sync_waits(mod))


bass.Bass.to_json_bytes = _to_json_bytes_split

# Optionally let walrus dedup per-matmul LDWEIGHTS (its ldw-opt pass is
# hardcoded off in bass_utils).
if os.environ.get("BBCU_LDWOPT", "0") != "0":
    import concourse.bass_utils as _bu

    _orig_run_command = _bu.run_command

    def _run_command_ldwopt(argv, **kw):
        if isinstance(argv, list):
            argv = ["--enable-ldw-opt=true" if a == "--enable-ldw-opt=false"
                    else a for a in argv]
        return _orig_run_command(argv, **kw)

    _bu.run_command = _run_command_ldwopt

F32 = mybir.dt.float32
BF16 = mybir.dt.bfloat16
FP8 = mybir.dt.float8e4
NP_FP8 = ml_dtypes.float8_e4m3
NP_BF16 = ml_dtypes.bfloat16
AL = mybir.AluOpType

# consts column indices (S' = conv of 0/1 plane; scale_eff = 2*scale,
# pb0_eff = pb0 - scale*rowsum)
C_SC = 0      # scale_eff (prelu activation scale)
C_PB0 = 1     # pb0_eff   (prelu activation bias)
C_AL = 2      # prelu alpha
C_FIN = 3     # pr_bias1 - move0_bias (final add)
C_RS = 4      # (1-a)*scale_eff        (relu path: ACT scale)
C_RB = 5      # (1-a)*pb0_eff          (relu path: ACT bias)
C_VS = 6      # a*scale_eff            (relu path: stt scalar)
C_RF = 7      # a*pb0_eff + pr_bias1 - move0_bias (relu path: final add)
# +-1 encoding variants (for chunks whose sign runs on ACT as Sign)
C2_SC = 8     # scale
C2_PB0 = 9    # pb0
C2_RS = 10
C2_RB = 11
C2_VS = 12
C2_RF = 13
NCOL = 14

SLOT = 272      # bytes per row-pair slot (16-aligned, >= 258)
PAD = np.float32(0.5)  # pad value: contributes 2*0.5-1 = 0 after the fold


_PM_SET = frozenset(
    int(t) for t in os.environ.get("BBCU_PM", "").split(",") if t != "")


def _is_pm(k: int) -> bool:
    # blocks whose sign runs on ACT with +-1 encoding (offloads the DVE)
    return k in _PM_SET


def _build3(Bc: int, H: int, W: int, C: int, G: int, look: int,
            use_prelu: bool = True):
    """Per-core Bass module. Chunk = G rows; parity layout 64ch x 2 parities."""
    assert C == 64 and W == 256
    assert H % G == 0 and G % 4 == 0
    P = G // 2             # row-pairs per chunk
    NCH = H // G           # chunks per image
    ABLK = P * SLOT        # 4352
    NCHT = Bc * NCH        # total chunks per core

    nc = bass.Bass()
    xd = nc.declare_dram_parameter("x", [NCHT, 128, P * W], BF16, isOutput=False)
    bd = nc.declare_dram_parameter("zb", [NCHT, 128, ABLK], FP8, isOutput=False)
    wd = nc.declare_dram_parameter("wp", [128, 768], FP8, isOutput=False)
    cd = nc.declare_dram_parameter("cv", [128, NCOL], F32, isOutput=False)
    cbd = nc.declare_dram_parameter("cvb", [128, NCOL], BF16, isOutput=False)
    yd = nc.declare_dram_parameter("y", [NCHT, 128, P * W], BF16, isOutput=True)

    with ExitStack() as ctx:
        tc = ctx.enter_context(tile.TileContext(nc))
        cpool = ctx.enter_context(tc.tile_pool(name="const", bufs=1))
        zpool = ctx.enter_context(tc.tile_pool(name="zs", bufs=1))
        XB = int(os.environ.get("BBCU_XB", "4"))
        xpool = ctx.enter_context(tc.tile_pool(name="xt", bufs=look + XB))
        gpool = ctx.enter_context(tc.tile_pool(name="gt", bufs=4 if G <= 32 else 2))
        g2pool = ctx.enter_context(tc.tile_pool(name="gt2", bufs=3))
        pspool = ctx.enter_context(tc.tile_pool(name="ps", bufs=2, space="PSUM"))

        # --- resident constants ---
        wsb = cpool.tile([128, 768], FP8)
        nc.sync.dma_start(wsb[:], wd[:])
        cvs = cpool.tile([128, NCOL], F32)
        nc.sync.dma_start(cvs[:], cd[:])
        cvb = cpool.tile([128, NCOL], BF16)
        nc.sync.dma_start(cvb[:], cbd[:])
        wv = wsb[:].rearrange("k (d z m) -> k d z m", d=3, m=128)

        # zs: per image-chunk k, [A block | B block]; A slot j = 0/1 plane of
        # row-pair kP+j (even rows parts 0:64, odd 64:128), B slot j = the
        # type-2 cross-pair rows (host-packed, halos and pads included).
        zsall = zpool.tile([128, NCH * 2 * ABLK], FP8)
        zv = zsall[:].rearrange("p (k ab s c) -> p k ab s c", ab=2, s=P, c=SLOT)

        # one-time A column pads (B comes fully padded from the host):
        # 0.5 for 0/1-encoded blocks, 0.0 for +-1-encoded blocks
        for k in range(NCH):
            pv = 0.0 if _is_pm(k) else float(PAD)
            nc.gpsimd.memset(zv[:, k, 0, :, 0:1], pv)
            nc.gpsimd.memset(zv[:, k, 0, :, 257:272], pv)

        # PE pstate warmup: dummy matmuls on a scratch tile while the first
        # chunk's loads are in flight (results discarded; first real matmul
        # clears its bank with start=True).
        WARM = int(os.environ.get("BBCU_WARM", "6"))

        def load(cc):
            xt = xpool.tile([128, P * W], BF16, name=f"xt_{cc}", tag="xt")
            nc.sync.dma_start(xt[:], xd[cc])
            k = cc % NCH
            nc.sync.dma_start(
                zsall[:, (2 * k + 1) * ABLK:(2 * k + 2) * ABLK], bd[cc])
            if cc == 0 and WARM:
                # PE pstate warmup just-in-time: reads chunk 0's B block, so
                # it starts only once that load lands (right before the first
                # real matmuls). Results are discarded (start=True clears).
                wps = pspool.tile([128, 2048], F32, name="ps")
                for wi in range(WARM):
                    nc.tensor.matmul(
                        wps[:, (wi % 8) * 256:(wi % 8 + 1) * 256],
                        wv[:, 0],
                        zv[:, 0, 1, 0:2, 0:256],
                        start=True, stop=True,
                        perf_mode=mybir.MatmulPerfMode.DoubleRow,
                    )
            return xt

        def sign(cc, xt):
            k = cc % NCH
            if _is_pm(k):
                nc.scalar.activation(
                    zv[:, k, 0, :, 1:257],
                    xt[:].rearrange("p (s c) -> p s c", c=W),
                    mybir.ActivationFunctionType.Sign)
            else:
                nc.vector.tensor_scalar(
                    zv[:, k, 0, :, 1:257],
                    xt[:].rearrange("p (s c) -> p s c", c=W),
                    0.0, None, op0=AL.is_ge)

        MW = int(os.environ.get("BBCU_MW", "2"))  # output slots per matmul
        LDW = os.environ.get("BBCU_LDW", "0") != "0"  # dedup ldweights

        def conv(cc, xt):
            k = cc % NCH
            gt = gpool.tile([128, P * W], BF16, name=f"gt_{cc}", tag="gt")
            NT = P // 8
            pss = [pspool.tile([128, 2048], F32, name="ps")
                   for t in range(NT)]
            if LDW:
                # dwi-major over the whole chunk: the first matmul of each
                # dwi run self-loads the stationary; the rest reuse it
                # (ldweights=False suppresses the per-matmul LDWEIGHTS).
                for dwi in range(3):
                    first = True
                    for t in range(NT):
                        for q in range(8 // MW):
                            j = 8 * t + MW * q
                            mm = nc.tensor.matmul(
                                pss[t][:, q * MW * 256:(q + 1) * MW * 256],
                                wv[:, dwi],
                                zv[:, k, :, j:j + MW, dwi:dwi + 256],
                                start=(dwi == 0),
                                stop=(dwi == 2),
                                perf_mode=mybir.MatmulPerfMode.DoubleRow,
                            )
                            if not first:
                                mm.ins.ldweights = False
                            first = False
            else:
                for t in range(NT):
                    ps = pss[t]
                    # dwi-outer so consecutive matmuls share weights;
                    # accumulation groups are per bank.
                    for dwi in range(3):
                        for q in range(8 // MW):
                            j = 8 * t + MW * q
                            nc.tensor.matmul(
                                ps[:, q * MW * 256:(q + 1) * MW * 256],
                                wv[:, dwi],
                                zv[:, k, :, j:j + MW, dwi:dwi + 256],
                                start=(dwi == 0),
                                stop=(dwi == 2),
                                perf_mode=mybir.MatmulPerfMode.DoubleRow,
                            )
            pm = _is_pm(k)
            for t in range(NT):
                ps = pss[t]
                gslice = gt[:, t * 2048:(t + 1) * 2048]
                csc = C2_SC if pm else C_SC
                cpb = C2_PB0 if pm else C_PB0
                crs = C2_RS if pm else C_RS
                crb = C2_RB if pm else C_RB
                cvsq = C2_VS if pm else C_VS
                if use_prelu:
                    nc.scalar.activation(
                        gslice,
                        ps[:],
                        mybir.ActivationFunctionType.Prelu,
                        bias=cvs[:, cpb:cpb + 1],
                        scale=cvs[:, csc:csc + 1],
                        alpha=cvs[:, C_AL:C_AL + 1],
                    )
                else:
                    rt = g2pool.tile([128, 2048], F32, name="rt", tag="rt")
                    nc.scalar.activation(
                        rt[:],
                        ps[:],
                        mybir.ActivationFunctionType.Relu,
                        bias=cvs[:, crb:crb + 1],
                        scale=cvs[:, crs:crs + 1],
                    )
                    nc.vector.scalar_tensor_tensor(
                        gslice, ps[:], cvs[:, cvsq:cvsq + 1], rt[:],
                        op0=AL.mult, op1=AL.add)
            # y = (g + cF) + x_dev  (one non-aliased DVE stt, one 1MB store)
            if use_prelu:
                ccol = C_FIN
            else:
                ccol = C2_RF if pm else C_RF
            gt2 = g2pool.tile([128, P * W], BF16, name=f"g2_{cc}", tag="g2")
            if os.environ.get("BBCU_STTH", "0") != "0" and NT == 2:
                # halves: the first can run while ACT handles tile 1
                for t in range(NT):
                    lo, hi = t * 2048, (t + 1) * 2048
                    nc.vector.scalar_tensor_tensor(
                        gt2[:, lo:hi], gt[:, lo:hi], cvb[:, ccol:ccol + 1],
                        xt[:, lo:hi], op0=AL.add, op1=AL.add)
            else:
                nc.vector.scalar_tensor_tensor(
                    gt2[:], gt[:], cvb[:, ccol:ccol + 1], xt[:],
                    op0=AL.add, op1=AL.add)
            # store from a non-sync queue: a store's semaphore wait on the
            # sync engine would head-of-line block later chunks' load issues
            sq = os.environ.get("BBCU_SQ", "scalar")
            getattr(nc, sq).dma_start(yd[cc], gt2[:])

        CONV_FIRST = os.environ.get("BBCU_CF", "0") != "0"
        xts = {}
        for idx in range(NCHT):
            xts[idx] = load(idx)
            if CONV_FIRST:
                if idx >= look:
                    conv(idx - look, xts.pop(idx - look))
                sign(idx, xts[idx])
            else:
                sign(idx, xts[idx])
                if idx >= look:
                    conv(idx - look, xts.pop(idx - look))
        for idx in sorted(xts):
            conv(idx, xts.pop(idx))

    return nc


def _host_prep3(move0_bias, conv_weight, prelu_weight, pr_bias0, pr_bias1):
    """Pack weights into [128, 3*2*128] fp8 lhsT + constant vectors."""
    w = np.asarray(conv_weight, dtype=np.float32)          # [O, I, 3, 3]
    sw = np.sign(w).astype(np.float32)
    scale = np.mean(np.abs(w), axis=(1, 2, 3)).astype(np.float32)  # [O]
    a = np.asarray(prelu_weight, dtype=np.float32).reshape(64)
    pb0 = np.asarray(pr_bias0, dtype=np.float32).reshape(64)
    pb1 = np.asarray(pr_bias1, dtype=np.float32).reshape(64)
    b0 = np.asarray(move0_bias, dtype=np.float32).reshape(64)

    # lhsT[k, m]: k = pi*64 + ci, m = po*64 + co -> sw[co, ci, kh, kw]
    # ktile 0 (type-1): dh = [[0, -1], [1, 0]][pi][po]
    # ktile 1 (type-2): (pi0,po1) = +1, (pi1,po0) = -1
    swT = np.transpose(sw, (1, 0, 2, 3))  # [ci, co, kh, kw]
    wdr = np.zeros((128, 3, 2, 128), dtype=np.float32)
    for idw in range(3):
        kw = idw
        wdr[0:64, idw, 0, 0:64] = swT[:, :, 1, kw]      # even->even  dh=0
        wdr[0:64, idw, 0, 64:128] = swT[:, :, 0, kw]    # even->odd   dh=-1
        wdr[64:128, idw, 0, 0:64] = swT[:, :, 2, kw]    # odd->even   dh=+1
        wdr[64:128, idw, 0, 64:128] = swT[:, :, 1, kw]  # odd->odd    dh=0
        wdr[0:64, idw, 1, 64:128] = swT[:, :, 2, kw]    # row 2i+2 -> out 2i+1
        wdr[64:128, idw, 1, 0:64] = swT[:, :, 0, kw]    # row 2i-1 -> out 2i
    wp8 = wdr.reshape(128, 768).astype(NP_FP8)

    # rowsum over the full K=256 contraction and all 3 dw taps, per out col m
    rowsum = wdr.sum(axis=(0, 1, 2))                    # [128]
    sc2 = np.concatenate([scale, scale])                # [128]
    sc_eff = 2.0 * sc2
    pb0_eff = np.concatenate([pb0, pb0]) - sc2 * rowsum

    cv = np.zeros((128, NCOL), dtype=np.float32)
    aa = np.concatenate([a, a])
    fin = np.concatenate([pb1 - b0, pb1 - b0])
    cv[:, C_SC] = sc_eff
    cv[:, C_PB0] = pb0_eff
    cv[:, C_AL] = aa
    cv[:, C_FIN] = fin
    cv[:, C_RS] = (1.0 - aa) * sc_eff
    cv[:, C_RB] = (1.0 - aa) * pb0_eff
    cv[:, C_VS] = aa * sc_eff
    cv[:, C_RF] = aa * pb0_eff + fin
    pb0_pm = np.concatenate([pb0, pb0])
    cv[:, C2_SC] = sc2
    cv[:, C2_PB0] = pb0_pm
    cv[:, C2_RS] = (1.0 - aa) * sc2
    cv[:, C2_RB] = (1.0 - aa) * pb0_pm
    cv[:, C2_VS] = aa * sc2
    cv[:, C2_RF] = aa * pb0_pm + fin
    return wp8, cv


def _pack_x(x, b0, G=32):
    """x [B,C,H,W] f32 -> [B, NCH, 128, (G/2)*W] bf16 of bf16(x + b0)."""
    B, C, H, W = x.shape
    P = G // 2
    NCH = H // G
    t = (x + b0.reshape(1, C, 1, 1)).astype(NP_BF16)
    v = t.reshape(B, C, NCH, P, 2, W).transpose(0, 2, 4, 1, 3, 5)
    return np.ascontiguousarray(v.reshape(B, NCH, 2 * C, P * W))


def _pack_zb(x, b0, G=32):
    """Host B plane: [B, NCH, 128, P*SLOT] fp8 of the shifted 0/1 plane.

    B slot j: even parts = 0/1 of even row of pair kP+j+1;
              odd parts  = 0/1 of odd row  of pair kP+j-1; halos/pads 0.5.
    """
    B, C, H, W = x.shape
    P = G // 2
    NCH = H // G
    NPAIR = H // 2
    z = ((x + b0.reshape(1, C, 1, 1)) >= 0).astype(np.float32)  # {0,1}
    zp = z.reshape(B, C, NPAIR, 2, W)  # [B, C, pair, parity, W]
    halo = np.full((B, C, 1, W), PAD, dtype=np.float32)
    even_sh = np.concatenate([zp[:, :, 1:, 0, :], halo], axis=2)   # pair+1 even
    odd_sh = np.concatenate([halo, zp[:, :, :-1, 1, :]], axis=2)   # pair-1 odd
    out = np.full((B, NCH, 2, C, P, SLOT), PAD, dtype=np.float32)
    ev = even_sh.reshape(B, C, NCH, P, W).transpose(0, 2, 1, 3, 4)
    od = odd_sh.reshape(B, C, NCH, P, W).transpose(0, 2, 1, 3, 4)
    out[:, :, 0, :, :, 1:257] = ev
    out[:, :, 1, :, :, 1:257] = od
    # +-1 encoded blocks: map {0,1,0.5(pad)} -> {-1,1,0}
    for k in range(NCH):
        if _is_pm(k):
            out[:, k] = 2.0 * out[:, k] - 1.0
    return np.ascontiguousarray(
        out.reshape(B, NCH, 128, P * SLOT).astype(NP_FP8))


def _unpack_y(yp, B, C, H, W, G=32):
    P = G // 2
    NCH = H // G
    v = yp.reshape(B, NCH, 2, C, P, W).transpose(0, 3, 1, 4, 2, 5)
    return v.reshape(B, C, H, W).astype(np.float32)


_NC_CACHE: dict = {}


def _get_nc3(key, *args):
    if key not in _NC_CACHE:
        _NC_CACHE[key] = _build3(*args)
    return _NC_CACHE[key]


def _make_in_maps(inputs, NCORES=8, G=32):
    x = np.asarray(inputs["x"], dtype=np.float32)
    B, C, H, W = x.shape
    Bc = B // NCORES
    NCH = H // G
    P = G // 2
    b0 = np.asarray(inputs["move0_bias"], dtype=np.float32)
    wp8, cv = _host_prep3(
        inputs["move0_bias"], inputs["conv_weight"], inputs["prelu_weight"],
        inputs["pr_bias0"], inputs["pr_bias1"])
    xp = _pack_x(x, b0, G)
    zb = _pack_zb(x, b0, G)
    in_maps = [
        {
            "x": np.ascontiguousarray(
                xp[i * Bc:(i + 1) * Bc].reshape(Bc * NCH, 128, P * W)),
            "zb": np.ascontiguousarray(
                zb[i * Bc:(i + 1) * Bc].reshape(Bc * NCH, 128, P * SLOT)),
            "wp": wp8,
            "cv": cv,
            "cvb": cv.astype(NP_BF16),
        }
        for i in range(NCORES)
    ]
    return in_maps, (B, C, H, W, Bc)


def kernel(x, move0_bias, conv_weight, prelu_weight, pr_bias0, pr_bias1):
    inputs = dict(x=x, move0_bias=move0_bias, conv_weight=conv_weight,
                  prelu_weight=prelu_weight, pr_bias0=pr_bias0,
                  pr_bias1=pr_bias1)
    NCORES = 8
    G = int(os.environ.get("BBCU_G", "32"))
    look = int(os.environ.get("BBCU_LOOK", "2" if G <= 32 else "1"))
    use_prelu = os.environ.get("BBCU_PRELU", "1") != "0"

    in_maps, (B, C, H, W, Bc) = _make_in_maps(inputs, NCORES, G)
    key = (Bc, H, W, C, G, look, use_prelu)
    nc = _get_nc3(key, Bc, H, W, C, G, look, use_prelu)

    res = run_bass_kernel_spmd(nc, in_maps, core_ids=list(range(NCORES)))
    yps = np.stack([res.results[i]["y"] for i in range(NCORES)], axis=0)
    return _unpack_y(yps, B, C, H, W, G)

